# revision 10
# baseline (speedup 1.0000x reference)
"""GAT-struc kernel for 8 Trainium2 NeuronCores (row-parallel attention).

Self-contained: hardcodes shapes/sharding for nn_GAT_struc (N=4096, NFEAT=512,
NHID=256, NHID_S=128, NHEADS=4, NLAYER=2, OUT_NHID_S=64), shards the node/row
dimension across 8 cores, runs one SPMD Bass program with on-device AllGathers
between GAT layers, and returns the full [4096, 64] output.
"""
import sys

sys.path.insert(0, "/opt/trn_rl_repo")

import numpy as np

import concourse.bacc as bacc
import concourse.bass as bass
import concourse.masks as masks
import concourse.mybir as mybir
import concourse.tile as tile

F32 = mybir.dt.float32
F16 = mybir.dt.float16
I32 = mybir.dt.int32
ALU = mybir.AluOpType
AF = mybir.ActivationFunctionType

# problem dims
N = 4096
NFEAT = 512       # == NFEAT_S
NHID = 256        # == OUT_NHID (attention hidden, both inner + output layers)
NHID_S = 128      # structural hidden (z cols, inner layers)
OUT_S = 64        # output structural hidden (z cols, output layer)
NHEADS = 4
ALPHA = 0.2

NC_ = 8           # cores
R = N // NC_      # 512 own rows per core
IT = R // 128     # 4 i-tiles
JC = N // 128     # 32 j-chunks
FC = NFEAT // 128  # 4 feature chunks
GRP = 4           # j-chunks per elementwise group (ACT op width 4*512=2048)

BIG = 60000.0     # additive mask magnitude (fp16-safe; lrelu*0.2 -> -12000 -> exp -> 0)

# attention units: u = 2*h + l for inner layers (h in 0..3, l in 0..1); u == 8 is output
UNITS = list(range(9))
L0_UNITS = [0, 2, 4, 6]   # (h, l=0)
L1_UNITS = [1, 3, 5, 7]   # (h, l=1)
U_OUT = 8


def build_program():
    nc = bacc.Bacc(
        "TRN2", target_bir_lowering=False, debug=False, num_devices=NC_
    )

    # ---- I/O ----
    xT = nc.dram_tensor("xT", [NFEAT, R], F32, kind="ExternalInput")
    xsT = nc.dram_tensor("xsT", [NFEAT, R], F32, kind="ExternalInput")
    adjT = nc.dram_tensor("adjT", [N, R], F32, kind="ExternalInput")
    obs = nc.dram_tensor("obs", [1, R], I32, kind="ExternalInput")
    theta = nc.dram_tensor("theta", [1, NFEAT], F32, kind="ExternalInput")
    W_d = nc.dram_tensor("W", [NHEADS, 2, NFEAT, NHID], F32, kind="ExternalInput")
    a_d = nc.dram_tensor("a", [NHEADS, 2, 2 * NHID], F32, kind="ExternalInput")
    Ws0_d = nc.dram_tensor("Ws0", [NHEADS, NFEAT, NHID_S], F32, kind="ExternalInput")
    Ws1_d = nc.dram_tensor("Ws1", [NHEADS, NHID_S, NHID_S], F32, kind="ExternalInput")
    Wo_d = nc.dram_tensor("Wo", [NFEAT, NHID], F32, kind="ExternalInput")
    ao_d = nc.dram_tensor("ao", [2 * NHID], F32, kind="ExternalInput")
    Wso_d = nc.dram_tensor("Wso", [NHEADS * NHID_S, OUT_S], F32, kind="ExternalInput")
    out_d = nc.dram_tensor("out", [R, OUT_S], F32, kind="ExternalOutput")

    # ---- internal DRAM (collective bounce buffers) ----
    dstb = nc.dram_tensor("dstb", [9, R], F32)
    dstg = nc.dram_tensor("dstg", [NC_, 9, R], F32, addr_space="Shared")
    zb0 = nc.dram_tensor("zb0", [R, NHEADS * NHID_S], F16)
    zg0 = nc.dram_tensor("zg0", [N, NHEADS * NHID_S], F16, addr_space="Shared")
    zb1 = nc.dram_tensor("zb1", [R, NHEADS * NHID_S], F16)
    zg1 = nc.dram_tensor("zg1", [N, NHEADS * NHID_S], F16, addr_space="Shared")
    zbo = nc.dram_tensor("zbo", [R, OUT_S], F16)
    zgo = nc.dram_tensor("zgo", [N, OUT_S], F16, addr_space="Shared")

    groups = [list(range(NC_))]

    with tile.TileContext(nc) as tc:
        with (
            tc.tile_pool(name="const", bufs=1) as constp,
            tc.tile_pool(name="xt", bufs=1) as xtp,
            tc.tile_pool(name="adjraw", bufs=2) as adjrawp,
            tc.tile_pool(name="adjm", bufs=1) as adjmp,
            tc.tile_pool(name="wsb", bufs=2) as wsbp,
            tc.tile_pool(name="wht", bufs=2) as whtp,
            tc.tile_pool(name="srcb", bufs=1) as srcbp,
            tc.tile_pool(name="small", bufs=2) as smallp,
            tc.tile_pool(name="zsb", bufs=1) as zsbp,
            tc.tile_pool(name="attt", bufs=5) as atttp,
            tc.tile_pool(name="hst", bufs=1) as hstp,
            tc.tile_pool(name="stage", bufs=4) as stagep,
            tc.tile_pool(name="post", bufs=4) as postp,
            tc.tile_pool(name="attps", bufs=1, space="PSUM") as attps,
            tc.tile_pool(name="miscps", bufs=2, space="PSUM") as miscps,
            tc.tile_pool(name="smallps", bufs=2, space="PSUM") as smallps,
        ):
            # ======== prep ========
            ident = constp.tile([128, 128], F32, tag="ident")
            masks.make_identity(nc, ident[:])

            theta_sb = constp.tile([128, FC], F32, tag="theta")
            nc.sync.dma_start(theta_sb[:], theta.ap()[0].rearrange("(c p) -> p c", p=128))

            obs_sb = constp.tile([1, R], I32, tag="obs")
            nc.sync.dma_start(obs_sb[:1, :], obs[:, :])
            seed = constp.tile([1, R], F32, tag="seed")
            nc.vector.tensor_scalar(seed[:1, :], obs_sb[:1, :], 1.0, None, ALU.is_equal)
            seedb = constp.tile([128, R], F32, tag="seedb")
            nc.gpsimd.partition_broadcast(seedb[:], seed[:1, :])

            # xT merged with seed * theta
            xt_sb = xtp.tile([128, FC * R], F32, tag="xt")
            nc.sync.dma_start(
                xt_sb[:].rearrange("p (c i) -> p c i", c=FC),
                xT.ap().rearrange("(c p) i -> p c i", p=128),
            )
            for fc in range(FC):
                sl = xt_sb[:, fc * R:(fc + 1) * R]
                nc.vector.scalar_tensor_tensor(
                    sl, seedb[:], theta_sb[:, fc:fc + 1], sl, ALU.mult, ALU.add
                )

            xst_sb = xtp.tile([128, FC * R], F32, tag="xst")
            nc.sync.dma_start(
                xst_sb[:].rearrange("p (c i) -> p c i", c=FC),
                xsT.ap().rearrange("(c p) i -> p c i", p=128),
            )

            # adjacency -> additive mask, fp16, transposed layout [j, i]
            adjm = adjmp.tile([128, JC * R], F16, tag="adjm")
            PIECE = 2  # j-chunks per conversion piece
            for pz in range(JC // PIECE):
                raw = adjrawp.tile([128, PIECE * R], F32, tag="adjraw")
                nc.sync.dma_start(
                    raw[:].rearrange("p (c i) -> p c i", c=PIECE),
                    adjT.ap()[pz * PIECE * 128:(pz + 1) * PIECE * 128, :].rearrange(
                        "(c p) i -> p c i", p=128
                    ),
                )
                nc.vector.tensor_scalar(
                    adjm[:, pz * PIECE * R:(pz + 1) * PIECE * R],
                    raw[:],
                    1.0,
                    BIG,
                    ALU.subtract,
                    ALU.mult,
                )

            # ======== per-unit linear prep: WhT -> srcT/dstT ========
            srcb = {}
            for u in UNITS:
                if u == U_OUT:
                    w_src = Wo_d.ap()
                    a_src = ao_d.ap()
                else:
                    h, l = u // 2, u % 2
                    w_src = W_d.ap()[h, l]
                    a_src = a_d.ap()[h, l]
                w_sb = wsbp.tile([128, FC * NHID], F32, tag="wsb")
                nc.sync.dma_start(
                    w_sb[:].rearrange("p (c m) -> p c m", c=FC),
                    w_src.rearrange("(c p) m -> p c m", p=128),
                )
                a_sb = smallp.tile([128, 4], F32, tag="asb")
                nc.sync.dma_start(a_sb[:], a_src.rearrange("(c p) -> p c", p=128))

                wht = whtp.tile([128, 2 * R], F32, tag="wht")
                for mh in range(2):
                    ps = miscps.tile([128, R], F32, tag="mp", name="whtps")
                    for fc in range(FC):
                        nc.tensor.matmul(
                            ps[:],
                            w_sb[:, fc * NHID + mh * 128: fc * NHID + (mh + 1) * 128],
                            xt_sb[:, fc * R:(fc + 1) * R],
                            start=(fc == 0),
                            stop=(fc == FC - 1),
                        )
                    nc.vector.tensor_copy(wht[:, mh * R:(mh + 1) * R], ps[:])

                sd_ps = smallps.tile([33, R], F32, tag="sp", name="sdps")
                for kc in range(2):
                    nc.tensor.matmul(
                        sd_ps[0:1, :],
                        a_sb[:, kc:kc + 1],
                        wht[:, kc * R:(kc + 1) * R],
                        start=(kc == 0),
                        stop=(kc == 1),
                    )
                for kc in range(2):
                    nc.tensor.matmul(
                        sd_ps[32:33, :],
                        a_sb[:, 2 + kc:3 + kc],
                        wht[:, kc * R:(kc + 1) * R],
                        start=(kc == 0),
                        stop=(kc == 1),
                    )
                src16 = smallp.tile([1, R], F16, tag="src16")
                nc.vector.tensor_copy(src16[:1, :], sd_ps[0:1, :])
                sb = srcbp.tile([128, R], F16, tag=f"srcb{u}")
                nc.gpsimd.partition_broadcast(sb[:], src16[:1, :])
                srcb[u] = sb
                dst32 = smallp.tile([1, R], F32, tag="dst32")
                nc.vector.tensor_copy(dst32[:1, :], sd_ps[32:33, :])
                nc.sync.dma_start(dstb.ap()[u:u + 1, :], dst32[:1, :])

            # gather dst across cores -> per-partition layout [128, 9*32]
            nc.gpsimd.collective_compute(
                "AllGather", ALU.bypass, replica_groups=groups,
                ins=[dstb.ap().opt()], outs=[dstg.ap().opt()],
            )
            # dst_full arrives partition-innermost; load c-major then PE-transpose
            dst_sb = constp.tile([128, 9 * JC], F32, tag="dst")
            for u in UNITS:
                dsttmp = smallp.tile([32, 128], F32, tag="dsttmp")
                for k in range(NC_):
                    nc.sync.dma_start(
                        dsttmp[k * 4:(k + 1) * 4, :],
                        dstg.ap()[k, u].rearrange("(r p) -> r p", p=128),
                    )
                tp9 = smallps.tile([128, 32], F32, tag="sp", name="dstps")
                nc.tensor.matmul(
                    tp9[:], dsttmp[:32, :], ident[:32, :32], is_transpose=True
                )
                nc.vector.tensor_copy(dst_sb[:, u * JC:(u + 1) * JC], tp9[:])

            # ======== helpers ========
            def build_z_slice(zg, col0, ncols_z, tag):
                """Gathered z [N, *] f16 slice -> SBUF [128, JC*(ncols_z+1)] with ones col."""
                zt = zsbp.tile([128, JC * (ncols_z + 1)], F16, tag=tag)
                view = zt[:].rearrange("p (c n) -> p c n", n=ncols_z + 1)
                nc.sync.dma_start(
                    view[:, :, 0:ncols_z],
                    zg.ap()[:, col0:col0 + ncols_z].rearrange(
                        "(c p) n -> p c n", p=128
                    ),
                )
                nc.vector.memset(view[:, :, ncols_z:ncols_z + 1], 1.0)
                return zt

            def attention(u, z_sb, ncols):
                """Returns 4 PSUM tiles [128, ncols+1]: att @ [z | 1] per i-tile."""
                nz = ncols + 1
                acc = [attps.tile([128, nz], F32, tag=f"attacc{it}", name=f"attacc{it}") for it in range(IT)]
                for g in range(JC // GRP):
                    tt = atttp.tile([128, GRP * R], F16, tag="attt")
                    for cc in range(GRP):
                        c = g * GRP + cc
                        sl = tt[:, cc * R:(cc + 1) * R]
                        nc.vector.scalar_tensor_tensor(
                            sl, srcb[u][:], dst_sb[:, u * JC + c:u * JC + c + 1],
                            adjm[:, c * R:(c + 1) * R], ALU.add, ALU.add,
                        )
                        nc.vector.scalar_tensor_tensor(
                            sl, sl, ALPHA, sl, ALU.mult, ALU.max
                        )
                    nc.scalar.activation(tt[:], tt[:], AF.Exp)
                    for cc in range(GRP):
                        c = g * GRP + cc
                        for it in range(IT):
                            nc.tensor.matmul(
                                acc[it][:],
                                tt[:, cc * R + it * 128: cc * R + (it + 1) * 128],
                                z_sb[:, c * nz:(c + 1) * nz],
                                start=(c == 0),
                                stop=(c == JC - 1),
                            )
                return acc

            def postprocess(acc, ncols, to_hsT=None, to_out=None):
                """normalize by denom col, ELU; either transpose into hsT tile or DMA out."""
                for it in range(IT):
                    rd = postp.tile([128, 1], F32, tag="rd")
                    nc.vector.reciprocal(rd[:], acc[it][:, ncols:ncols + 1])
                    hv = postp.tile([128, ncols], F32, tag="hv")
                    nc.vector.tensor_scalar(
                        hv[:], acc[it][:, 0:ncols], rd[:, 0:1], None, ALU.mult
                    )
                    mn = postp.tile([128, ncols], F32, tag="mn")
                    nc.vector.tensor_scalar(mn[:], hv[:], 0.0, None, ALU.min)
                    nc.scalar.activation(mn[:], mn[:], AF.Exp)
                    elu = postp.tile([128, ncols], F32, tag="elu")
                    nc.vector.scalar_tensor_tensor(
                        elu[:], mn[:], -1.0, hv[:], ALU.add, ALU.max
                    )
                    if to_out is not None:
                        nc.sync.dma_start(
                            to_out.ap()[it * 128:(it + 1) * 128, :], elu[:]
                        )
                    else:
                        tp = smallps.tile([128, 128], F32, tag="sp", name="trps")
                        nc.tensor.matmul(tp[:], elu[:], ident[:], is_transpose=True)
                        nc.vector.tensor_copy(
                            to_hsT[:, it * 128:(it + 1) * 128], tp[:]
                        )

            # ======== z0 ========
            ws0_sb = wsbp.tile([128, NHEADS * FC * NHID_S], F32, tag="ws0")
            nc.sync.dma_start(
                ws0_sb[:].rearrange("p (h c n) -> p h c n", h=NHEADS, c=FC),
                Ws0_d.ap().rearrange("h (c p) n -> p h c n", p=128),
            )
            for h in range(NHEADS):
                for it in range(IT):
                    ps = miscps.tile([128, NHID_S], F32, tag="mp", name="zps")
                    for fc in range(FC):
                        nc.tensor.matmul(
                            ps[:],
                            xst_sb[:, fc * R + it * 128: fc * R + (it + 1) * 128],
                            ws0_sb[:, (h * FC + fc) * NHID_S:(h * FC + fc + 1) * NHID_S],
                            start=(fc == 0),
                            stop=(fc == FC - 1),
                        )
                    st = stagep.tile([128, NHID_S], F16, tag="zstage")
                    nc.vector.tensor_copy(st[:], ps[:])
                    nc.sync.dma_start(
                        zb0.ap()[it * 128:(it + 1) * 128, h * NHID_S:(h + 1) * NHID_S],
                        st[:],
                    )
            nc.gpsimd.collective_compute(
                "AllGather", ALU.bypass, replica_groups=groups,
                ins=[zb0.ap().opt()], outs=[zg0.ap().opt()],
            )

            # ======== layer 0 attention ========
            hsT = {}
            for h in range(NHEADS):
                z_sb = build_z_slice(zg0, h * NHID_S, NHID_S, tag=f"z{h}")
                u = 2 * h + 0
                acc = attention(u, z_sb, NHID_S)
                ht = hstp.tile([128, R], F32, tag=f"hsT{h}")
                postprocess(acc, NHID_S, to_hsT=ht)
                hsT[h] = ht

            # ======== z1 ========
            ws1_sb = wsbp.tile([128, NHEADS * NHID_S], F32, tag="ws1")
            nc.sync.dma_start(
                ws1_sb[:].rearrange("p (h n) -> p h n", h=NHEADS),
                Ws1_d.ap().rearrange("h p n -> p h n"),
            )
            for h in range(NHEADS):
                for it in range(IT):
                    ps = miscps.tile([128, NHID_S], F32, tag="mp", name="zps")
                    nc.tensor.matmul(
                        ps[:],
                        hsT[h][:, it * 128:(it + 1) * 128],
                        ws1_sb[:, h * NHID_S:(h + 1) * NHID_S],
                        start=True,
                        stop=True,
                    )
                    st = stagep.tile([128, NHID_S], F16, tag="zstage")
                    nc.vector.tensor_copy(st[:], ps[:])
                    nc.sync.dma_start(
                        zb1.ap()[it * 128:(it + 1) * 128, h * NHID_S:(h + 1) * NHID_S],
                        st[:],
                    )
            nc.gpsimd.collective_compute(
                "AllGather", ALU.bypass, replica_groups=groups,
                ins=[zb1.ap().opt()], outs=[zg1.ap().opt()],
            )

            # ======== layer 1 attention ========
            for h in range(NHEADS):
                z_sb = build_z_slice(zg1, h * NHID_S, NHID_S, tag=f"z{h}")
                u = 2 * h + 1
                acc = attention(u, z_sb, NHID_S)
                ht = hstp.tile([128, R], F32, tag=f"hsT{h}")
                postprocess(acc, NHID_S, to_hsT=ht)
                hsT[h] = ht

            # ======== zo ========
            wso_sb = wsbp.tile([128, NHEADS * OUT_S], F32, tag="wso")
            nc.sync.dma_start(
                wso_sb[:].rearrange("p (h n) -> p h n", h=NHEADS),
                Wso_d.ap().rearrange("(h p) n -> p h n", p=128),
            )
            for it in range(IT):
                ps = miscps.tile([128, OUT_S], F32, tag="mp", name="zops")
                for h in range(NHEADS):
                    nc.tensor.matmul(
                        ps[:],
                        hsT[h][:, it * 128:(it + 1) * 128],
                        wso_sb[:, h * OUT_S:(h + 1) * OUT_S],
                        start=(h == 0),
                        stop=(h == NHEADS - 1),
                    )
                st = stagep.tile([128, OUT_S], F16, tag="zostage")
                nc.vector.tensor_copy(st[:], ps[:])
                nc.sync.dma_start(zbo.ap()[it * 128:(it + 1) * 128, :], st[:])
            nc.gpsimd.collective_compute(
                "AllGather", ALU.bypass, replica_groups=groups,
                ins=[zbo.ap().opt()], outs=[zgo.ap().opt()],
            )

            # ======== output attention ========
            z_sb = build_z_slice(zgo, 0, OUT_S, tag="z0")
            acc = attention(U_OUT, z_sb, OUT_S)
            postprocess(acc, OUT_S, to_out=out_d)

    nc.compile()
    return nc


_CACHE = {}


def _get_nc():
    if "nc" not in _CACHE:
        _CACHE["nc"] = build_program()
    return _CACHE["nc"]


def make_in_maps(x, adj, observation, x_struc, theta, W, a, Ws0, Ws1, Wo, ao, Wso):
    x = np.asarray(x, np.float32)
    adj = np.asarray(adj, np.float32)
    x_struc = np.asarray(x_struc, np.float32)
    in_maps = []
    for k in range(NC_):
        rows = slice(k * R, (k + 1) * R)
        in_maps.append({
            "xT": np.ascontiguousarray(x[rows].T),
            "xsT": np.ascontiguousarray(x_struc[rows].T),
            "adjT": np.ascontiguousarray(adj[rows].T),
            "obs": np.ascontiguousarray(np.asarray(observation, np.int32)[:, rows]),
            "theta": np.asarray(theta, np.float32),
            "W": np.asarray(W, np.float32),
            "a": np.asarray(a, np.float32),
            "Ws0": np.asarray(Ws0, np.float32),
            "Ws1": np.asarray(Ws1, np.float32),
            "Wo": np.asarray(Wo, np.float32),
            "ao": np.asarray(ao, np.float32),
            "Wso": np.asarray(Wso, np.float32),
        })
    return in_maps


def kernel(**inputs) -> np.ndarray:
    from concourse.bass_utils import run_bass_kernel_spmd

    nc = _get_nc()
    in_maps = make_in_maps(**inputs)
    res = run_bass_kernel_spmd(nc, in_maps, list(range(NC_)))
    out = np.concatenate([res.results[k]["out"] for k in range(NC_)], axis=0)
    return out.astype(np.float32)


# revision 15
# speedup vs baseline: 1.4266x; 1.4266x over previous
"""GAT-struc kernel for 8 Trainium2 NeuronCores (row-parallel attention).

Self-contained: hardcodes shapes/sharding for nn_GAT_struc (N=4096, NFEAT=512,
NHID=256, NHID_S=128, NHEADS=4, NLAYER=2, OUT_NHID_S=64), shards the node/row
dimension across 8 cores, runs one SPMD Bass program with on-device AllGathers
between GAT layers, and returns the full [4096, 64] output.
"""
import sys

sys.path.insert(0, "/opt/trn_rl_repo")

import numpy as np

import concourse.bacc as bacc
import concourse.bass as bass
import concourse.masks as masks
import concourse.mybir as mybir
import concourse.tile as tile

F32 = mybir.dt.float32
F32R = mybir.dt.float32r
F16 = mybir.dt.float16
I32 = mybir.dt.int32


def r32(ap):
    return ap.bitcast(F32R)
ALU = mybir.AluOpType
AF = mybir.ActivationFunctionType

# problem dims
N = 4096
NFEAT = 512       # == NFEAT_S
NHID = 256        # == OUT_NHID (attention hidden, both inner + output layers)
NHID_S = 128      # structural hidden (z cols, inner layers)
OUT_S = 64        # output structural hidden (z cols, output layer)
NHEADS = 4
ALPHA = 0.2

NC_ = 8           # cores
R = N // NC_      # 512 own rows per core
IT = R // 128     # 4 i-tiles
JC = N // 128     # 32 j-chunks
FC = NFEAT // 128  # 4 feature chunks
GRP = 4           # j-chunks per elementwise group (ACT op width 4*512=2048)

BIG = 60000.0     # additive mask magnitude (fp16-safe; lrelu*0.2 -> -12000 -> exp -> 0)

# attention units: u = 2*h + l for inner layers (h in 0..3, l in 0..1); u == 8 is output
UNITS = list(range(9))
L0_UNITS = [0, 2, 4, 6]   # (h, l=0)
L1_UNITS = [1, 3, 5, 7]   # (h, l=1)
U_OUT = 8


def build_program():
    nc = bacc.Bacc(
        "TRN2", target_bir_lowering=False, debug=False, num_devices=NC_
    )

    # ---- I/O ----
    xT = nc.dram_tensor("xT", [NFEAT, R], F32, kind="ExternalInput")
    xsT = nc.dram_tensor("xsT", [NFEAT, R], F32, kind="ExternalInput")
    adjT = nc.dram_tensor("adjT", [N, R], F32, kind="ExternalInput")
    obs = nc.dram_tensor("obs", [1, R], I32, kind="ExternalInput")
    theta = nc.dram_tensor("theta", [1, NFEAT], F32, kind="ExternalInput")
    W_d = nc.dram_tensor("W", [NHEADS, 2, NFEAT, NHID], F32, kind="ExternalInput")
    a_d = nc.dram_tensor("a", [NHEADS, 2, 2 * NHID], F32, kind="ExternalInput")
    Ws0_d = nc.dram_tensor("Ws0", [NHEADS, NFEAT, NHID_S], F32, kind="ExternalInput")
    Ws1_d = nc.dram_tensor("Ws1", [NHEADS, NHID_S, NHID_S], F32, kind="ExternalInput")
    Wo_d = nc.dram_tensor("Wo", [NFEAT, NHID], F32, kind="ExternalInput")
    ao_d = nc.dram_tensor("ao", [2 * NHID], F32, kind="ExternalInput")
    Wso_d = nc.dram_tensor("Wso", [NHEADS * NHID_S, OUT_S], F32, kind="ExternalInput")
    out_d = nc.dram_tensor("out", [R, OUT_S], F32, kind="ExternalOutput")

    # ---- internal DRAM (collective bounce buffers) ----
    dstb = nc.dram_tensor("dstb", [9, R], F32)
    dstg = nc.dram_tensor("dstg", [NC_, 9, R], F32, addr_space="Shared")
    zb0 = nc.dram_tensor("zb0", [R, NHEADS * NHID_S], F16)
    zg0 = nc.dram_tensor("zg0", [N, NHEADS * NHID_S], F16, addr_space="Shared")
    zb1 = nc.dram_tensor("zb1", [R, NHEADS * NHID_S], F16)
    zg1 = nc.dram_tensor("zg1", [N, NHEADS * NHID_S], F16, addr_space="Shared")
    zbo = nc.dram_tensor("zbo", [R, OUT_S], F16)
    zgo = nc.dram_tensor("zgo", [N, OUT_S], F16, addr_space="Shared")

    groups = [list(range(NC_))]

    with tile.TileContext(nc) as tc:
        with (
            tc.tile_pool(name="const", bufs=1) as constp,
            tc.tile_pool(name="xt", bufs=1) as xtp,
            tc.tile_pool(name="adjraw", bufs=2) as adjrawp,
            tc.tile_pool(name="adjm", bufs=1) as adjmp,
            tc.tile_pool(name="wsb", bufs=2) as wsbp,
            tc.tile_pool(name="wht", bufs=2) as whtp,
            tc.tile_pool(name="srcb", bufs=1) as srcbp,
            tc.tile_pool(name="small", bufs=2) as smallp,
            tc.tile_pool(name="zsb", bufs=1) as zsbp,
            tc.tile_pool(name="zv", bufs=2) as zvp,
            tc.tile_pool(name="attt", bufs=5) as atttp,
            tc.tile_pool(name="hst", bufs=1) as hstp,
            tc.tile_pool(name="stage", bufs=4) as stagep,
            tc.tile_pool(name="post", bufs=4) as postp,
            tc.tile_pool(name="attps", bufs=1, space="PSUM") as attps,
            tc.tile_pool(name="miscps", bufs=2, space="PSUM") as miscps,
            tc.tile_pool(name="smallps", bufs=2, space="PSUM") as smallps,
        ):
            # ======== prep ========
            ident = constp.tile([128, 128], F32, tag="ident")
            masks.make_identity(nc, ident[:])

            theta_sb = constp.tile([128, FC], F32, tag="theta")
            nc.sync.dma_start(theta_sb[:], theta.ap()[0].rearrange("(c p) -> p c", p=128))

            obs_sb = constp.tile([1, R], I32, tag="obs")
            nc.sync.dma_start(obs_sb[:1, :], obs[:, :])
            seed = constp.tile([1, R], F32, tag="seed")
            nc.vector.tensor_scalar(seed[:1, :], obs_sb[:1, :], 1.0, None, ALU.is_equal)
            seedb = constp.tile([128, R], F32, tag="seedb")
            nc.gpsimd.partition_broadcast(seedb[:], seed[:1, :])

            # xT merged with seed * theta
            xt_sb = xtp.tile([128, FC * R], F32R, tag="xt")
            nc.sync.dma_start(
                xt_sb[:].rearrange("p (c i) -> p c i", c=FC),
                r32(xT.ap()).rearrange("(c p) i -> p c i", p=128),
            )
            for fc in range(FC):
                sl = xt_sb[:, fc * R:(fc + 1) * R]
                nc.vector.scalar_tensor_tensor(
                    sl, seedb[:], theta_sb[:, fc:fc + 1], sl, ALU.mult, ALU.add
                )

            xst_sb = xtp.tile([128, FC * R], F32R, tag="xst")
            nc.sync.dma_start(
                xst_sb[:].rearrange("p (c i) -> p c i", c=FC),
                r32(xsT.ap()).rearrange("(c p) i -> p c i", p=128),
            )

            # adjacency 0/1 mask, fp16, transposed layout [j, i]
            adjm = adjmp.tile([128, JC * R], F16, tag="adjm")
            PIECE = 2  # j-chunks per conversion piece
            for pz in range(JC // PIECE):
                raw = adjrawp.tile([128, PIECE * R], F32, tag="adjraw")
                nc.sync.dma_start(
                    raw[:].rearrange("p (c i) -> p c i", c=PIECE),
                    adjT.ap()[pz * PIECE * 128:(pz + 1) * PIECE * 128, :].rearrange(
                        "(c p) i -> p c i", p=128
                    ),
                )
                nc.vector.tensor_scalar(
                    adjm[:, pz * PIECE * R:(pz + 1) * PIECE * R],
                    raw[:],
                    1.0,
                    None,
                    ALU.mult,
                )

            # ======== per-unit linear prep: WhT -> srcT/dstT ========
            srcb = {}
            for u in UNITS:
                if u == U_OUT:
                    w_src = Wo_d.ap()
                    a_src = ao_d.ap()
                else:
                    h, l = u // 2, u % 2
                    w_src = W_d.ap()[h, l]
                    a_src = a_d.ap()[h, l]
                w_sb = wsbp.tile([128, FC * NHID], F32R, tag="wsb")
                nc.sync.dma_start(
                    w_sb[:].rearrange("p (c m) -> p c m", c=FC),
                    r32(w_src).rearrange("(c p) m -> p c m", p=128),
                )
                a_sb = smallp.tile([128, 4], F32R, tag="asb")
                nc.sync.dma_start(a_sb[:], r32(a_src).rearrange("(c p) -> p c", p=128))

                wht = whtp.tile([128, 2 * R], F32R, tag="wht")
                for mh in range(2):
                    ps = miscps.tile([128, R], F32, tag="mp", name="whtps")
                    for fc in range(FC):
                        nc.tensor.matmul(
                            ps[:],
                            r32(w_sb[:, fc * NHID + mh * 128: fc * NHID + (mh + 1) * 128]),
                            r32(xt_sb[:, fc * R:(fc + 1) * R]),
                            start=(fc == 0),
                            stop=(fc == FC - 1),
                        )
                    nc.vector.tensor_copy(wht[:, mh * R:(mh + 1) * R], ps[:])

                src_ps = smallps.tile([1, R], F32, tag="sp", name="srcps")
                for kc in range(2):
                    nc.tensor.matmul(
                        src_ps[0:1, :],
                        r32(a_sb[:, kc:kc + 1]),
                        r32(wht[:, kc * R:(kc + 1) * R]),
                        start=(kc == 0),
                        stop=(kc == 1),
                    )
                dst_ps = smallps.tile([1, R], F32, tag="sp", name="dstps")
                for kc in range(2):
                    nc.tensor.matmul(
                        dst_ps[0:1, :],
                        r32(a_sb[:, 2 + kc:3 + kc]),
                        r32(wht[:, kc * R:(kc + 1) * R]),
                        start=(kc == 0),
                        stop=(kc == 1),
                    )
                src16 = smallp.tile([1, R], F16, tag="src16")
                nc.scalar.activation(src16[:1, :], src_ps[0:1, :], AF.Exp, scale=0.8)
                sb = srcbp.tile([128, R], F16, tag=f"srcb{u}")
                nc.gpsimd.partition_broadcast(sb[:], src16[:1, :])
                srcb[u] = sb
                dst32 = smallp.tile([1, R], F32, tag="dst32")
                nc.vector.tensor_copy(dst32[:1, :], dst_ps[0:1, :])
                nc.sync.dma_start(dstb.ap()[u:u + 1, :], dst32[:1, :])

            # gather dst across cores -> per-partition layout [128, 9*32]
            nc.gpsimd.collective_compute(
                "AllGather", ALU.bypass, replica_groups=groups,
                ins=[dstb.ap().opt()], outs=[dstg.ap().opt()],
            )
            # dst_full arrives partition-innermost; load c-major then PE-transpose
            dst_sb = constp.tile([128, 9 * JC], F32, tag="dst")
            for u in UNITS:
                dsttmp = smallp.tile([32, 128], F32, tag="dsttmp")
                for k in range(NC_):
                    nc.sync.dma_start(
                        dsttmp[k * 4:(k + 1) * 4, :],
                        dstg.ap()[k, u].rearrange("(r p) -> r p", p=128),
                    )
                tp9 = smallps.tile([128, 32], F32, tag="sp", name="dstps")
                nc.tensor.matmul(
                    tp9[:], dsttmp[:32, :], ident[:32, :32], is_transpose=True
                )
                nc.vector.tensor_copy(dst_sb[:, u * JC:(u + 1) * JC], tp9[:])
            q_sb = constp.tile([128, 9 * JC], F32, tag="qsb")
            nc.scalar.activation(q_sb[:], dst_sb[:], AF.Exp, scale=0.8)
            b_sb = constp.tile([128, 9 * JC], F32, tag="bsb")
            nc.scalar.activation(b_sb[:], dst_sb[:], AF.Exp, scale=0.2)

            # ======== helpers ========
            def build_z_slice(zg, col0, ncols_z, tag):
                """Gathered z [N, *] f16 slice -> SBUF [128, JC*(ncols_z+1)] with ones col."""
                zt = zsbp.tile([128, JC * (ncols_z + 1)], F16, tag=tag)
                view = zt[:].rearrange("p (c n) -> p c n", n=ncols_z + 1)
                nc.sync.dma_start(
                    view[:, :, 0:ncols_z],
                    zg.ap()[:, col0:col0 + ncols_z].rearrange(
                        "(c p) n -> p c n", p=128
                    ),
                )
                nc.vector.memset(view[:, :, ncols_z:ncols_z + 1], 1.0)
                return zt

            def attention(u, z_sb, ncols):
                """att @ [z | 1] per i-tile via the factored form:
                W = a_i*b_j*max(1, p_i*q_j); a_i cancels; b_j folded into zv."""
                nz = ncols + 1
                # zv = diag(b) @ [z | 1]
                zv = zvp.tile([128, JC * nz], F16, tag="zv")
                for c in range(JC):
                    nc.vector.tensor_scalar(
                        zv[:, c * nz:(c + 1) * nz],
                        z_sb[:, c * nz:(c + 1) * nz],
                        b_sb[:, u * JC + c:u * JC + c + 1],
                        None,
                        ALU.mult,
                    )
                acc = [attps.tile([128, nz], F32, tag=f"attacc{it}", name=f"attacc{it}") for it in range(IT)]
                for g in range(JC // GRP):
                    tt = atttp.tile([128, GRP * R], F16, tag="attt")
                    for cc in range(GRP):
                        c = g * GRP + cc
                        sl = tt[:, cc * R:(cc + 1) * R]
                        nc.vector.tensor_scalar(
                            sl, srcb[u][:], q_sb[:, u * JC + c:u * JC + c + 1],
                            1.0, ALU.mult, ALU.max,
                        )
                    nc.vector.tensor_tensor(
                        tt[:], tt[:], adjm[:, g * GRP * R:(g + 1) * GRP * R], ALU.mult
                    )
                    for cc in range(GRP):
                        c = g * GRP + cc
                        for it in range(IT):
                            nc.tensor.matmul(
                                acc[it][:],
                                tt[:, cc * R + it * 128: cc * R + (it + 1) * 128],
                                zv[:, c * nz:(c + 1) * nz],
                                start=(c == 0),
                                stop=(c == JC - 1),
                            )
                return acc

            def postprocess(acc, ncols, to_hsT=None, to_out=None):
                """normalize by denom col, ELU; either transpose into hsT tile or DMA out."""
                for it in range(IT):
                    rd = postp.tile([128, 1], F32, tag="rd")
                    nc.vector.reciprocal(rd[:], acc[it][:, ncols:ncols + 1])
                    hv = postp.tile([128, ncols], F32, tag="hv")
                    nc.vector.tensor_scalar(
                        hv[:], acc[it][:, 0:ncols], rd[:, 0:1], None, ALU.mult
                    )
                    mn = postp.tile([128, ncols], F32, tag="mn")
                    nc.vector.tensor_scalar(mn[:], hv[:], 0.0, None, ALU.min)
                    nc.scalar.activation(mn[:], mn[:], AF.Exp)
                    elu = postp.tile([128, ncols], F32, tag="elu")
                    nc.vector.scalar_tensor_tensor(
                        elu[:], mn[:], -1.0, hv[:], ALU.add, ALU.max
                    )
                    if to_out is not None:
                        nc.sync.dma_start(
                            to_out.ap()[it * 128:(it + 1) * 128, :], elu[:]
                        )
                    else:
                        tp = smallps.tile([128, 128], F32, tag="sp", name="trps")
                        nc.tensor.matmul(tp[:], elu[:], ident[:], is_transpose=True)
                        nc.vector.tensor_copy(
                            to_hsT[:, it * 128:(it + 1) * 128], tp[:]
                        )

            # ======== z0 ========
            ws0_sb = wsbp.tile([128, NHEADS * FC * NHID_S], F32R, tag="ws0")
            nc.sync.dma_start(
                ws0_sb[:].rearrange("p (h c n) -> p h c n", h=NHEADS, c=FC),
                r32(Ws0_d.ap()).rearrange("h (c p) n -> p h c n", p=128),
            )
            for h in range(NHEADS):
                for it in range(IT):
                    ps = miscps.tile([128, NHID_S], F32, tag="mp", name="zps")
                    for fc in range(FC):
                        nc.tensor.matmul(
                            ps[:],
                            r32(xst_sb[:, fc * R + it * 128: fc * R + (it + 1) * 128]),
                            r32(ws0_sb[:, (h * FC + fc) * NHID_S:(h * FC + fc + 1) * NHID_S]),
                            start=(fc == 0),
                            stop=(fc == FC - 1),
                        )
                    st = stagep.tile([128, NHID_S], F16, tag="zstage")
                    nc.vector.tensor_copy(st[:], ps[:])
                    nc.sync.dma_start(
                        zb0.ap()[it * 128:(it + 1) * 128, h * NHID_S:(h + 1) * NHID_S],
                        st[:],
                    )
            nc.gpsimd.collective_compute(
                "AllGather", ALU.bypass, replica_groups=groups,
                ins=[zb0.ap().opt()], outs=[zg0.ap().opt()],
            )

            # ======== layer 0 attention ========
            hsT = {}
            for h in range(NHEADS):
                z_sb = build_z_slice(zg0, h * NHID_S, NHID_S, tag=f"z{h}")
                u = 2 * h + 0
                acc = attention(u, z_sb, NHID_S)
                ht = hstp.tile([128, R], F32R, tag=f"hsT{h}")
                postprocess(acc, NHID_S, to_hsT=ht)
                hsT[h] = ht

            # ======== z1 ========
            ws1_sb = wsbp.tile([128, NHEADS * NHID_S], F32R, tag="ws1")
            nc.sync.dma_start(
                ws1_sb[:].rearrange("p (h n) -> p h n", h=NHEADS),
                r32(Ws1_d.ap()).rearrange("h p n -> p h n"),
            )
            for h in range(NHEADS):
                for it in range(IT):
                    ps = miscps.tile([128, NHID_S], F32, tag="mp", name="zps")
                    nc.tensor.matmul(
                        ps[:],
                        r32(hsT[h][:, it * 128:(it + 1) * 128]),
                        r32(ws1_sb[:, h * NHID_S:(h + 1) * NHID_S]),
                        start=True,
                        stop=True,
                    )
                    st = stagep.tile([128, NHID_S], F16, tag="zstage")
                    nc.vector.tensor_copy(st[:], ps[:])
                    nc.sync.dma_start(
                        zb1.ap()[it * 128:(it + 1) * 128, h * NHID_S:(h + 1) * NHID_S],
                        st[:],
                    )
            nc.gpsimd.collective_compute(
                "AllGather", ALU.bypass, replica_groups=groups,
                ins=[zb1.ap().opt()], outs=[zg1.ap().opt()],
            )

            # ======== layer 1 attention ========
            for h in range(NHEADS):
                z_sb = build_z_slice(zg1, h * NHID_S, NHID_S, tag=f"z{h}")
                u = 2 * h + 1
                acc = attention(u, z_sb, NHID_S)
                ht = hstp.tile([128, R], F32R, tag=f"hsT{h}")
                postprocess(acc, NHID_S, to_hsT=ht)
                hsT[h] = ht

            # ======== zo ========
            wso_sb = wsbp.tile([128, NHEADS * OUT_S], F32R, tag="wso")
            nc.sync.dma_start(
                wso_sb[:].rearrange("p (h n) -> p h n", h=NHEADS),
                r32(Wso_d.ap()).rearrange("(h p) n -> p h n", p=128),
            )
            for it in range(IT):
                ps = miscps.tile([128, OUT_S], F32, tag="mp", name="zops")
                for h in range(NHEADS):
                    nc.tensor.matmul(
                        ps[:],
                        r32(hsT[h][:, it * 128:(it + 1) * 128]),
                        r32(wso_sb[:, h * OUT_S:(h + 1) * OUT_S]),
                        start=(h == 0),
                        stop=(h == NHEADS - 1),
                    )
                st = stagep.tile([128, OUT_S], F16, tag="zostage")
                nc.vector.tensor_copy(st[:], ps[:])
                nc.sync.dma_start(zbo.ap()[it * 128:(it + 1) * 128, :], st[:])
            nc.gpsimd.collective_compute(
                "AllGather", ALU.bypass, replica_groups=groups,
                ins=[zbo.ap().opt()], outs=[zgo.ap().opt()],
            )

            # ======== output attention ========
            z_sb = build_z_slice(zgo, 0, OUT_S, tag="z0")
            acc = attention(U_OUT, z_sb, OUT_S)
            postprocess(acc, OUT_S, to_out=out_d)

    nc.compile()
    return nc


_CACHE = {}


def _get_nc():
    if "nc" not in _CACHE:
        _CACHE["nc"] = build_program()
    return _CACHE["nc"]


def make_in_maps(x, adj, observation, x_struc, theta, W, a, Ws0, Ws1, Wo, ao, Wso):
    x = np.asarray(x, np.float32)
    adj = np.asarray(adj, np.float32)
    x_struc = np.asarray(x_struc, np.float32)
    in_maps = []
    for k in range(NC_):
        rows = slice(k * R, (k + 1) * R)
        in_maps.append({
            "xT": np.ascontiguousarray(x[rows].T),
            "xsT": np.ascontiguousarray(x_struc[rows].T),
            "adjT": np.ascontiguousarray(adj[rows].T),
            "obs": np.ascontiguousarray(np.asarray(observation, np.int32)[:, rows]),
            "theta": np.asarray(theta, np.float32),
            "W": np.asarray(W, np.float32),
            "a": np.asarray(a, np.float32),
            "Ws0": np.asarray(Ws0, np.float32),
            "Ws1": np.asarray(Ws1, np.float32),
            "Wo": np.asarray(Wo, np.float32),
            "ao": np.asarray(ao, np.float32),
            "Wso": np.asarray(Wso, np.float32),
        })
    return in_maps


def kernel(**inputs) -> np.ndarray:
    from concourse.bass_utils import run_bass_kernel_spmd

    nc = _get_nc()
    in_maps = make_in_maps(**inputs)
    res = run_bass_kernel_spmd(nc, in_maps, list(range(NC_)))
    out = np.concatenate([res.results[k]["out"] for k in range(NC_)], axis=0)
    return out.astype(np.float32)


# revision 16
# speedup vs baseline: 1.5371x; 1.0774x over previous
"""GAT-struc kernel for 8 Trainium2 NeuronCores (row-parallel attention).

Self-contained: hardcodes shapes/sharding for nn_GAT_struc (N=4096, NFEAT=512,
NHID=256, NHID_S=128, NHEADS=4, NLAYER=2, OUT_NHID_S=64), shards the node/row
dimension across 8 cores, runs one SPMD Bass program with on-device AllGathers
between GAT layers, and returns the full [4096, 64] output.

Attention is computed in transposed layout [j, i] per 128-row j-chunk using the
factored form  exp(lrelu(src_i + dst_j)) = a_i * b_j * max(1, p_i * q_j)  with
a = e^{0.2 src} (cancels in softmax), b = e^{0.2 dst} (folded into z rows),
p = e^{0.8 src}, q = e^{0.8 dst} — so the N^2 work is one tensor_scalar (4x)
plus one masked tensor_tensor (2x) on DVE, and the matmul's ones-column gives
the softmax denominator for free.
"""
import sys

sys.path.insert(0, "/opt/trn_rl_repo")

import numpy as np

import concourse.bacc as bacc
import concourse.bass as bass
import concourse.masks as masks
import concourse.mybir as mybir
import concourse.tile as tile

F32 = mybir.dt.float32
F32R = mybir.dt.float32r
F16 = mybir.dt.float16
I32 = mybir.dt.int32


def r32(ap):
    return ap.bitcast(F32R)


ALU = mybir.AluOpType
AF = mybir.ActivationFunctionType

# problem dims
N = 4096
NFEAT = 512       # == NFEAT_S
NHID = 256        # == OUT_NHID (attention hidden, both inner + output layers)
NHID_S = 128      # structural hidden (z cols, inner layers)
OUT_S = 64        # output structural hidden (z cols, output layer)
NHEADS = 4
ALPHA = 0.2

NC_ = 8           # cores
R = N // NC_      # 512 own rows per core
IT = R // 128     # 4 i-tiles
JC = N // 128     # 32 j-chunks
FC = NFEAT // 128  # 4 feature chunks
GRP = 8           # j-chunks per masked-TT group

# attention units: u = 2*h + l for inner layers; u == 8 is output
UNITS = list(range(9))
U_OUT = 8


def build_program():
    nc = bacc.Bacc(
        "TRN2", target_bir_lowering=False, debug=False, num_devices=NC_
    )

    # ---- I/O ----
    xT = nc.dram_tensor("xT", [NFEAT, R], F32, kind="ExternalInput")
    xsT = nc.dram_tensor("xsT", [NFEAT, R], F32, kind="ExternalInput")
    adjT = nc.dram_tensor("adjT", [N, R], F32, kind="ExternalInput")
    obs = nc.dram_tensor("obs", [1, R], I32, kind="ExternalInput")
    theta = nc.dram_tensor("theta", [1, NFEAT], F32, kind="ExternalInput")
    W_d = nc.dram_tensor("W", [NHEADS, 2, NFEAT, NHID], F32, kind="ExternalInput")
    a_d = nc.dram_tensor("a", [NHEADS, 2, 2 * NHID], F32, kind="ExternalInput")
    Ws0_d = nc.dram_tensor("Ws0", [NHEADS, NFEAT, NHID_S], F32, kind="ExternalInput")
    Ws1_d = nc.dram_tensor("Ws1", [NHEADS, NHID_S, NHID_S], F32, kind="ExternalInput")
    Wo_d = nc.dram_tensor("Wo", [NFEAT, NHID], F32, kind="ExternalInput")
    ao_d = nc.dram_tensor("ao", [2 * NHID], F32, kind="ExternalInput")
    Wso_d = nc.dram_tensor("Wso", [NHEADS * NHID_S, OUT_S], F32, kind="ExternalInput")
    out_d = nc.dram_tensor("out", [R, OUT_S], F32, kind="ExternalOutput")

    # ---- internal DRAM (collective bounce buffers, per head for pipelining) ----
    dstb = nc.dram_tensor("dstb", [9, R], F32)
    dstg = nc.dram_tensor("dstg", [NC_, 9, R], F32, addr_space="Shared")
    zb0 = [nc.dram_tensor(f"zb0_{h}", [R, NHID_S], F16) for h in range(NHEADS)]
    zg0 = [
        nc.dram_tensor(f"zg0_{h}", [N, NHID_S], F16, addr_space="Shared")
        for h in range(NHEADS)
    ]
    zb1 = [nc.dram_tensor(f"zb1_{h}", [R, NHID_S], F16) for h in range(NHEADS)]
    zg1 = [
        nc.dram_tensor(f"zg1_{h}", [N, NHID_S], F16, addr_space="Shared")
        for h in range(NHEADS)
    ]
    zbo = nc.dram_tensor("zbo", [R, OUT_S], F16)
    zgo = nc.dram_tensor("zgo", [N, OUT_S], F16, addr_space="Shared")

    groups = [list(range(NC_))]

    with tile.TileContext(nc) as tc:
        with (
            tc.tile_pool(name="const", bufs=1) as constp,
            tc.tile_pool(name="xt", bufs=1) as xtp,
            tc.tile_pool(name="adjraw", bufs=2) as adjrawp,
            tc.tile_pool(name="adjm", bufs=1) as adjmp,
            tc.tile_pool(name="wsb", bufs=2) as wsbp,
            tc.tile_pool(name="wht", bufs=2) as whtp,
            tc.tile_pool(name="srcb", bufs=1) as srcbp,
            tc.tile_pool(name="small", bufs=2) as smallp,
            tc.tile_pool(name="zsb", bufs=2) as zsbp,
            tc.tile_pool(name="zv", bufs=2) as zvp,
            tc.tile_pool(name="attt", bufs=3) as atttp,
            tc.tile_pool(name="hst", bufs=1) as hstp,
            tc.tile_pool(name="stage", bufs=4) as stagep,
            tc.tile_pool(name="post", bufs=4) as postp,
            tc.tile_pool(name="attps", bufs=1, space="PSUM") as attps,
            tc.tile_pool(name="miscps", bufs=2, space="PSUM") as miscps,
            tc.tile_pool(name="smallps", bufs=2, space="PSUM") as smallps,
        ):
            # ======== constants + x/xs loads ========
            ident = constp.tile([128, 128], F32, tag="ident")
            masks.make_identity(nc, ident[:])

            theta_sb = constp.tile([128, FC], F32, tag="theta")
            nc.sync.dma_start(theta_sb[:], theta.ap()[0].rearrange("(c p) -> p c", p=128))

            obs_sb = constp.tile([1, R], I32, tag="obs")
            nc.sync.dma_start(obs_sb[:1, :], obs[:, :])
            seed = constp.tile([1, R], F32, tag="seed")
            nc.vector.tensor_scalar(seed[:1, :], obs_sb[:1, :], 1.0, None, ALU.is_equal)
            seedb = constp.tile([128, R], F32, tag="seedb")
            nc.gpsimd.partition_broadcast(seedb[:], seed[:1, :])

            xst_sb = xtp.tile([128, FC * R], F32R, tag="xst")
            nc.sync.dma_start(
                xst_sb[:].rearrange("p (c i) -> p c i", c=FC),
                r32(xsT.ap()).rearrange("(c p) i -> p c i", p=128),
            )

            xt_sb = xtp.tile([128, FC * R], F32R, tag="xt")
            nc.sync.dma_start(
                xt_sb[:].rearrange("p (c i) -> p c i", c=FC),
                r32(xT.ap()).rearrange("(c p) i -> p c i", p=128),
            )
            for fc in range(FC):
                sl = xt_sb[:, fc * R:(fc + 1) * R]
                nc.vector.scalar_tensor_tensor(
                    sl, seedb[:], theta_sb[:, fc:fc + 1], sl, ALU.mult, ALU.add
                )

            # ======== z0 (head-pipelined: compute + gather ASAP) ========
            ws0_sb = wsbp.tile([128, NHEADS * FC * NHID_S], F32R, tag="ws0")
            nc.sync.dma_start(
                ws0_sb[:].rearrange("p (h c n) -> p h c n", h=NHEADS, c=FC),
                r32(Ws0_d.ap()).rearrange("h (c p) n -> p h c n", p=128),
            )
            for h in range(NHEADS):
                for it in range(IT):
                    ps = miscps.tile([128, NHID_S], F32, tag="mp", name="zps")
                    for fc in range(FC):
                        nc.tensor.matmul(
                            ps[:],
                            r32(xst_sb[:, fc * R + it * 128: fc * R + (it + 1) * 128]),
                            r32(ws0_sb[:, (h * FC + fc) * NHID_S:(h * FC + fc + 1) * NHID_S]),
                            start=(fc == 0),
                            stop=(fc == FC - 1),
                        )
                    st = stagep.tile([128, NHID_S], F16, tag="zstage")
                    nc.scalar.copy(st[:], ps[:])
                    nc.sync.dma_start(
                        zb0[h].ap()[it * 128:(it + 1) * 128, :], st[:]
                    )
                nc.gpsimd.collective_compute(
                    "AllGather", ALU.bypass, replica_groups=groups,
                    ins=[zb0[h].ap().opt()], outs=[zg0[h].ap().opt()],
                )

            # ======== adjacency 0/1 mask, fp16, transposed layout [j, i] ========
            adjm = adjmp.tile([128, JC * R], F16, tag="adjm")
            PIECE = 2
            for pz in range(JC // PIECE):
                raw = adjrawp.tile([128, PIECE * R], F32, tag="adjraw")
                nc.sync.dma_start(
                    raw[:].rearrange("p (c i) -> p c i", c=PIECE),
                    adjT.ap()[pz * PIECE * 128:(pz + 1) * PIECE * 128, :].rearrange(
                        "(c p) i -> p c i", p=128
                    ),
                )
                nc.vector.tensor_scalar(
                    adjm[:, pz * PIECE * R:(pz + 1) * PIECE * R],
                    raw[:], 1.0, None, ALU.mult,
                )

            # ======== per-unit linear prep: WhT -> srcT/dstT ========
            srcb = {}
            for u in UNITS:
                if u == U_OUT:
                    w_src = Wo_d.ap()
                    a_src = ao_d.ap()
                else:
                    h, l = u // 2, u % 2
                    w_src = W_d.ap()[h, l]
                    a_src = a_d.ap()[h, l]
                w_sb = wsbp.tile([128, FC * NHID], F32R, tag="wsb")
                nc.sync.dma_start(
                    w_sb[:].rearrange("p (c m) -> p c m", c=FC),
                    r32(w_src).rearrange("(c p) m -> p c m", p=128),
                )
                a_sb = smallp.tile([128, 4], F32R, tag="asb")
                nc.sync.dma_start(a_sb[:], r32(a_src).rearrange("(c p) -> p c", p=128))

                wht = whtp.tile([128, 2 * R], F32R, tag="wht")
                for mh in range(2):
                    ps = miscps.tile([128, R], F32, tag="mp", name="whtps")
                    for fc in range(FC):
                        nc.tensor.matmul(
                            ps[:],
                            r32(w_sb[:, fc * NHID + mh * 128: fc * NHID + (mh + 1) * 128]),
                            r32(xt_sb[:, fc * R:(fc + 1) * R]),
                            start=(fc == 0),
                            stop=(fc == FC - 1),
                        )
                    nc.scalar.copy(wht[:, mh * R:(mh + 1) * R], ps[:])

                src_ps = smallps.tile([1, R], F32, tag="sp", name="srcps")
                for kc in range(2):
                    nc.tensor.matmul(
                        src_ps[0:1, :],
                        r32(a_sb[:, kc:kc + 1]),
                        r32(wht[:, kc * R:(kc + 1) * R]),
                        start=(kc == 0),
                        stop=(kc == 1),
                    )
                dst_ps = smallps.tile([1, R], F32, tag="sp", name="dstps")
                for kc in range(2):
                    nc.tensor.matmul(
                        dst_ps[0:1, :],
                        r32(a_sb[:, 2 + kc:3 + kc]),
                        r32(wht[:, kc * R:(kc + 1) * R]),
                        start=(kc == 0),
                        stop=(kc == 1),
                    )
                # p_i = exp(0.8 * src_i), broadcast along partitions
                src16 = smallp.tile([1, R], F16, tag="src16")
                nc.scalar.activation(src16[:1, :], src_ps[0:1, :], AF.Exp, scale=0.8)
                sb = srcbp.tile([128, R], F16, tag=f"srcb{u}")
                nc.gpsimd.partition_broadcast(sb[:], src16[:1, :])
                srcb[u] = sb
                dst32 = smallp.tile([1, R], F32, tag="dst32")
                nc.scalar.copy(dst32[:1, :], dst_ps[0:1, :])
                nc.sync.dma_start(dstb.ap()[u:u + 1, :], dst32[:1, :])

            # gather dst; arrives partition-innermost -> load c-major + PE-transpose
            nc.gpsimd.collective_compute(
                "AllGather", ALU.bypass, replica_groups=groups,
                ins=[dstb.ap().opt()], outs=[dstg.ap().opt()],
            )
            dst_sb = constp.tile([128, 9 * JC], F32, tag="dst")
            for u in UNITS:
                dsttmp = smallp.tile([32, 128], F32, tag="dsttmp")
                for k in range(NC_):
                    nc.sync.dma_start(
                        dsttmp[k * 4:(k + 1) * 4, :],
                        dstg.ap()[k, u].rearrange("(r p) -> r p", p=128),
                    )
                tp9 = smallps.tile([128, 32], F32, tag="sp", name="dstps2")
                nc.tensor.matmul(
                    tp9[:], dsttmp[:32, :], ident[:32, :32], is_transpose=True
                )
                nc.scalar.copy(dst_sb[:, u * JC:(u + 1) * JC], tp9[:])
            # q_j = exp(0.8 dst_j), b_j = exp(0.2 dst_j)
            q_sb = constp.tile([128, 9 * JC], F32, tag="qsb")
            nc.scalar.activation(q_sb[:], dst_sb[:], AF.Exp, scale=0.8)
            b_sb = constp.tile([128, 9 * JC], F32, tag="bsb")
            nc.scalar.activation(b_sb[:], dst_sb[:], AF.Exp, scale=0.2)

            # ======== helpers ========
            def build_z_slice(zg, ncols_z, tag):
                """Gathered z [N, ncols_z] f16 -> SBUF [128, JC*(ncols_z+1)] + ones col."""
                zt = zsbp.tile([128, JC * (ncols_z + 1)], F16, tag=tag, name=f"zsb_{tag}")
                view = zt[:].rearrange("p (c n) -> p c n", n=ncols_z + 1)
                nc.sync.dma_start(
                    view[:, :, 0:ncols_z],
                    zg.ap().rearrange("(c p) n -> p c n", p=128),
                )
                nc.vector.memset(view[:, :, ncols_z:ncols_z + 1], 1.0)
                return zt

            def attention(u, z_sb, ncols):
                """acc[it] = attT_chunks(u).T @ [b*z | b] accumulated over chunks."""
                nz = ncols + 1
                # zv = diag(b) @ [z | 1]  (on ACT: copy with per-partition scale)
                zv = zvp.tile([128, JC * nz], F16, tag="zv")
                for c in range(JC):
                    nc.scalar.activation(
                        zv[:, c * nz:(c + 1) * nz],
                        z_sb[:, c * nz:(c + 1) * nz],
                        AF.Copy, bias=0.0,
                        scale=b_sb[:, u * JC + c:u * JC + c + 1],
                    )
                acc = [
                    attps.tile([128, nz], F32, tag=f"attacc{it}", name=f"attacc{it}")
                    for it in range(IT)
                ]
                for g in range(JC // GRP):
                    tt = atttp.tile([128, GRP * R], F16, tag="attt")
                    for cc in range(GRP):
                        c = g * GRP + cc
                        sl = tt[:, cc * R:(cc + 1) * R]
                        nc.vector.tensor_scalar(
                            sl, srcb[u][:], q_sb[:, u * JC + c:u * JC + c + 1],
                            1.0, ALU.mult, ALU.max,
                        )
                    nc.vector.tensor_tensor(
                        tt[:], tt[:], adjm[:, g * GRP * R:(g + 1) * GRP * R], ALU.mult
                    )
                    for cc in range(GRP):
                        c = g * GRP + cc
                        for it in range(IT):
                            nc.tensor.matmul(
                                acc[it][:],
                                tt[:, cc * R + it * 128: cc * R + (it + 1) * 128],
                                zv[:, c * nz:(c + 1) * nz],
                                start=(c == 0),
                                stop=(c == JC - 1),
                            )
                return acc

            def postprocess(acc, ncols, to_hsT=None, to_out=None):
                """hs = elu(num/den);  elu(v) = max(exp(-relu(-v)) - 1, v)."""
                for it in range(IT):
                    rd = postp.tile([128, 1], F32, tag="rd")
                    nc.vector.reciprocal(rd[:], acc[it][:, ncols:ncols + 1])
                    hv = postp.tile([128, ncols], F32, tag="hv")
                    nc.vector.tensor_scalar(
                        hv[:], acc[it][:, 0:ncols], rd[:, 0:1], None, ALU.mult
                    )
                    mn = postp.tile([128, ncols], F32, tag="mn")
                    nc.scalar.activation(mn[:], hv[:], AF.Relu, scale=-1.0)
                    nc.scalar.activation(mn[:], mn[:], AF.Exp, scale=-1.0)
                    elu = postp.tile([128, ncols], F32, tag="elu")
                    nc.vector.scalar_tensor_tensor(
                        elu[:], mn[:], -1.0, hv[:], ALU.add, ALU.max
                    )
                    if to_out is not None:
                        nc.sync.dma_start(
                            to_out.ap()[it * 128:(it + 1) * 128, :], elu[:]
                        )
                    else:
                        tp = smallps.tile([128, 128], F32, tag="sp", name="trps")
                        nc.tensor.matmul(tp[:], elu[:], ident[:], is_transpose=True)
                        nc.scalar.copy(to_hsT[:, it * 128:(it + 1) * 128], tp[:])

            # ======== layer 0 attention (+ pipelined z1 per head) ========
            ws1_sb = wsbp.tile([128, NHEADS * NHID_S], F32R, tag="ws1")
            nc.sync.dma_start(
                ws1_sb[:].rearrange("p (h n) -> p h n", h=NHEADS),
                r32(Ws1_d.ap()).rearrange("h p n -> p h n"),
            )
            hsT = {}
            for h in range(NHEADS):
                z_sb = build_z_slice(zg0[h], NHID_S, tag="z")
                acc = attention(2 * h, z_sb, NHID_S)
                ht = hstp.tile([128, R], F32R, tag=f"hsT{h}", name=f"hsT{h}")
                postprocess(acc, NHID_S, to_hsT=ht)
                hsT[h] = ht
                # z1[h] = hs0[h] @ Ws1[h] -> bounce -> gather (overlaps next heads)
                for it in range(IT):
                    ps = miscps.tile([128, NHID_S], F32, tag="mp", name="zps1")
                    nc.tensor.matmul(
                        ps[:],
                        r32(ht[:, it * 128:(it + 1) * 128]),
                        r32(ws1_sb[:, h * NHID_S:(h + 1) * NHID_S]),
                        start=True, stop=True,
                    )
                    st = stagep.tile([128, NHID_S], F16, tag="zstage")
                    nc.scalar.copy(st[:], ps[:])
                    nc.sync.dma_start(
                        zb1[h].ap()[it * 128:(it + 1) * 128, :], st[:]
                    )
                nc.gpsimd.collective_compute(
                    "AllGather", ALU.bypass, replica_groups=groups,
                    ins=[zb1[h].ap().opt()], outs=[zg1[h].ap().opt()],
                )

            # ======== layer 1 attention ========
            for h in range(NHEADS):
                z_sb = build_z_slice(zg1[h], NHID_S, tag="z")
                acc = attention(2 * h + 1, z_sb, NHID_S)
                ht = hstp.tile([128, R], F32R, tag=f"hsT{h}", name=f"hsT1{h}")
                postprocess(acc, NHID_S, to_hsT=ht)
                hsT[h] = ht

            # ======== zo ========
            wso_sb = wsbp.tile([128, NHEADS * OUT_S], F32R, tag="wso")
            nc.sync.dma_start(
                wso_sb[:].rearrange("p (h n) -> p h n", h=NHEADS),
                r32(Wso_d.ap()).rearrange("(h p) n -> p h n", p=128),
            )
            for it in range(IT):
                ps = miscps.tile([128, OUT_S], F32, tag="mp", name="zops")
                for h in range(NHEADS):
                    nc.tensor.matmul(
                        ps[:],
                        r32(hsT[h][:, it * 128:(it + 1) * 128]),
                        r32(wso_sb[:, h * OUT_S:(h + 1) * OUT_S]),
                        start=(h == 0),
                        stop=(h == NHEADS - 1),
                    )
                st = stagep.tile([128, OUT_S], F16, tag="zostage")
                nc.scalar.copy(st[:], ps[:])
                nc.sync.dma_start(zbo.ap()[it * 128:(it + 1) * 128, :], st[:])
            nc.gpsimd.collective_compute(
                "AllGather", ALU.bypass, replica_groups=groups,
                ins=[zbo.ap().opt()], outs=[zgo.ap().opt()],
            )

            # ======== output attention ========
            z_sb = build_z_slice(zgo, OUT_S, tag="z")
            acc = attention(U_OUT, z_sb, OUT_S)
            postprocess(acc, OUT_S, to_out=out_d)

    nc.compile()
    return nc


_CACHE = {}


def _get_nc():
    if "nc" not in _CACHE:
        _CACHE["nc"] = build_program()
    return _CACHE["nc"]


def make_in_maps(x, adj, observation, x_struc, theta, W, a, Ws0, Ws1, Wo, ao, Wso):
    x = np.asarray(x, np.float32)
    adj = np.asarray(adj, np.float32)
    x_struc = np.asarray(x_struc, np.float32)
    in_maps = []
    for k in range(NC_):
        rows = slice(k * R, (k + 1) * R)
        in_maps.append({
            "xT": np.ascontiguousarray(x[rows].T),
            "xsT": np.ascontiguousarray(x_struc[rows].T),
            "adjT": np.ascontiguousarray(adj[rows].T),
            "obs": np.ascontiguousarray(np.asarray(observation, np.int32)[:, rows]),
            "theta": np.asarray(theta, np.float32),
            "W": np.asarray(W, np.float32),
            "a": np.asarray(a, np.float32),
            "Ws0": np.asarray(Ws0, np.float32),
            "Ws1": np.asarray(Ws1, np.float32),
            "Wo": np.asarray(Wo, np.float32),
            "ao": np.asarray(ao, np.float32),
            "Wso": np.asarray(Wso, np.float32),
        })
    return in_maps


def kernel(**inputs) -> np.ndarray:
    from concourse.bass_utils import run_bass_kernel_spmd

    nc = _get_nc()
    in_maps = make_in_maps(**inputs)
    res = run_bass_kernel_spmd(nc, in_maps, list(range(NC_)))
    out = np.concatenate([res.results[k]["out"] for k in range(NC_)], axis=0)
    return out.astype(np.float32)


# revision 17
# speedup vs baseline: 1.6684x; 1.0854x over previous
"""GAT-struc kernel for 8 Trainium2 NeuronCores (row-parallel attention).

Self-contained: hardcodes shapes/sharding for nn_GAT_struc (N=4096, NFEAT=512,
NHID=256, NHID_S=128, NHEADS=4, NLAYER=2, OUT_NHID_S=64), shards the node/row
dimension across 8 cores, runs one SPMD Bass program with on-device AllGathers
between GAT layers, and returns the full [4096, 64] output.

Attention is computed in transposed layout [j, i] per 128-row j-chunk using the
factored form  exp(lrelu(src_i + dst_j)) = a_i * b_j * max(1, p_i * q_j)  with
a = e^{0.2 src} (cancels in softmax), b = e^{0.2 dst} (folded into z rows),
p = e^{0.8 src}, q = e^{0.8 dst} — so the N^2 work is one tensor_scalar (4x)
plus one masked tensor_tensor (2x) on DVE, and the matmul's ones-column gives
the softmax denominator for free.
"""
import sys

sys.path.insert(0, "/opt/trn_rl_repo")

import numpy as np

import concourse.bacc as bacc
import concourse.bass as bass
import concourse.masks as masks
import concourse.mybir as mybir
import concourse.tile as tile

F32 = mybir.dt.float32
F32R = mybir.dt.float32r
F16 = mybir.dt.float16
I32 = mybir.dt.int32


def r32(ap):
    return ap.bitcast(F32R)


ALU = mybir.AluOpType
AF = mybir.ActivationFunctionType

# problem dims
N = 4096
NFEAT = 512       # == NFEAT_S
NHID = 256        # == OUT_NHID (attention hidden, both inner + output layers)
NHID_S = 128      # structural hidden (z cols, inner layers)
OUT_S = 64        # output structural hidden (z cols, output layer)
NHEADS = 4
ALPHA = 0.2

NC_ = 8           # cores
R = N // NC_      # 512 own rows per core
IT = R // 128     # 4 i-tiles
JC = N // 128     # 32 j-chunks
FC = NFEAT // 128  # 4 feature chunks
GRP = 8           # j-chunks per masked-TT group

# attention units: u = 2*h + l for inner layers; u == 8 is output
UNITS = list(range(9))
U_OUT = 8


def build_program():
    nc = bacc.Bacc(
        "TRN2", target_bir_lowering=False, debug=False, num_devices=NC_
    )

    # ---- I/O ----
    xT = nc.dram_tensor("xT", [NFEAT, R], F32, kind="ExternalInput")
    xsT = nc.dram_tensor("xsT", [NFEAT, R], F32, kind="ExternalInput")
    adjT = nc.dram_tensor("adjT", [N, R], F32, kind="ExternalInput")
    obs = nc.dram_tensor("obs", [1, R], I32, kind="ExternalInput")
    theta = nc.dram_tensor("theta", [1, NFEAT], F32, kind="ExternalInput")
    W_d = nc.dram_tensor("W", [NHEADS, 2, NFEAT, NHID], F32, kind="ExternalInput")
    a_d = nc.dram_tensor("a", [NHEADS, 2, 2 * NHID], F32, kind="ExternalInput")
    Ws0_d = nc.dram_tensor("Ws0", [NHEADS, NFEAT, NHID_S], F32, kind="ExternalInput")
    Ws1_d = nc.dram_tensor("Ws1", [NHEADS, NHID_S, NHID_S], F32, kind="ExternalInput")
    Wo_d = nc.dram_tensor("Wo", [NFEAT, NHID], F32, kind="ExternalInput")
    ao_d = nc.dram_tensor("ao", [2 * NHID], F32, kind="ExternalInput")
    Wso_d = nc.dram_tensor("Wso", [NHEADS * NHID_S, OUT_S], F32, kind="ExternalInput")
    out_d = nc.dram_tensor("out", [R, OUT_S], F32, kind="ExternalOutput")

    # ---- internal DRAM (collective bounce buffers, per head for pipelining) ----
    dstb = nc.dram_tensor("dstb", [9, R], F32)
    dstg = nc.dram_tensor("dstg", [NC_, 9, R], F32, addr_space="Shared")
    zb0 = nc.dram_tensor("zb0", [R, NHEADS * NHID_S], F16)
    zg0 = nc.dram_tensor("zg0", [N, NHEADS * NHID_S], F16, addr_space="Shared")
    zb1 = [nc.dram_tensor(f"zb1_{h}", [R, NHID_S], F16) for h in range(NHEADS)]
    zg1 = [
        nc.dram_tensor(f"zg1_{h}", [N, NHID_S], F16, addr_space="Shared")
        for h in range(NHEADS)
    ]
    zbo = nc.dram_tensor("zbo", [R, OUT_S], F16)
    zgo = nc.dram_tensor("zgo", [N, OUT_S], F16, addr_space="Shared")

    groups = [list(range(NC_))]

    with tile.TileContext(nc) as tc:
        with (
            tc.tile_pool(name="const", bufs=1) as constp,
            tc.tile_pool(name="xt", bufs=1) as xtp,
            tc.tile_pool(name="adjraw", bufs=2) as adjrawp,
            tc.tile_pool(name="adjm", bufs=1) as adjmp,
            tc.tile_pool(name="wsb", bufs=2) as wsbp,
            tc.tile_pool(name="wht", bufs=2) as whtp,
            tc.tile_pool(name="srcb", bufs=1) as srcbp,
            tc.tile_pool(name="small", bufs=2) as smallp,
            tc.tile_pool(name="zsb", bufs=2) as zsbp,
            tc.tile_pool(name="zv", bufs=2) as zvp,
            tc.tile_pool(name="attt", bufs=4) as atttp,
            tc.tile_pool(name="hst", bufs=1) as hstp,
            tc.tile_pool(name="stage", bufs=4) as stagep,
            tc.tile_pool(name="post", bufs=4) as postp,
            tc.tile_pool(name="attps", bufs=1, space="PSUM") as attps,
            tc.tile_pool(name="miscps", bufs=2, space="PSUM") as miscps,
            tc.tile_pool(name="smallps", bufs=2, space="PSUM") as smallps,
        ):
            # ======== constants + x/xs loads ========
            ident = constp.tile([128, 128], F32, tag="ident")
            masks.make_identity(nc, ident[:])

            theta_sb = constp.tile([128, FC], F32, tag="theta")
            nc.sync.dma_start(theta_sb[:], theta.ap()[0].rearrange("(c p) -> p c", p=128))

            obs_sb = constp.tile([1, R], I32, tag="obs")
            nc.sync.dma_start(obs_sb[:1, :], obs[:, :])
            seed = constp.tile([1, R], F32, tag="seed")
            nc.vector.tensor_scalar(seed[:1, :], obs_sb[:1, :], 1.0, None, ALU.is_equal)
            seedb = constp.tile([128, R], F32, tag="seedb")
            nc.gpsimd.partition_broadcast(seedb[:], seed[:1, :])

            xst_sb = xtp.tile([128, FC * R], F32R, tag="xst")
            nc.sync.dma_start(
                xst_sb[:].rearrange("p (c i) -> p c i", c=FC),
                r32(xsT.ap()).rearrange("(c p) i -> p c i", p=128),
            )

            xt_sb = xtp.tile([128, FC * R], F32R, tag="xt")
            nc.sync.dma_start(
                xt_sb[:].rearrange("p (c i) -> p c i", c=FC),
                r32(xT.ap()).rearrange("(c p) i -> p c i", p=128),
            )
            for fc in range(FC):
                sl = xt_sb[:, fc * R:(fc + 1) * R]
                nc.vector.scalar_tensor_tensor(
                    sl, seedb[:], theta_sb[:, fc:fc + 1], sl, ALU.mult, ALU.add
                )

            # ======== adjacency 0/1 mask, fp16, transposed layout [j, i] ========
            adjm = adjmp.tile([128, JC * R], F16, tag="adjm")
            PIECE = 2
            for pz in range(JC // PIECE):
                raw = adjrawp.tile([128, PIECE * R], F32, tag="adjraw")
                nc.sync.dma_start(
                    raw[:].rearrange("p (c i) -> p c i", c=PIECE),
                    adjT.ap()[pz * PIECE * 128:(pz + 1) * PIECE * 128, :].rearrange(
                        "(c p) i -> p c i", p=128
                    ),
                )
                nc.vector.tensor_scalar(
                    adjm[:, pz * PIECE * R:(pz + 1) * PIECE * R],
                    raw[:], 1.0, None, ALU.mult,
                )

            # ======== per-unit linear prep: WhT -> srcT/dstT ========
            srcb = {}
            for u in UNITS:
                if u == U_OUT:
                    w_src = Wo_d.ap()
                    a_src = ao_d.ap()
                else:
                    h, l = u // 2, u % 2
                    w_src = W_d.ap()[h, l]
                    a_src = a_d.ap()[h, l]
                w_sb = wsbp.tile([128, FC * NHID], F32R, tag="wsb")
                nc.sync.dma_start(
                    w_sb[:].rearrange("p (c m) -> p c m", c=FC),
                    r32(w_src).rearrange("(c p) m -> p c m", p=128),
                )
                a_sb = smallp.tile([128, 4], F32R, tag="asb")
                nc.sync.dma_start(a_sb[:], r32(a_src).rearrange("(c p) -> p c", p=128))

                wht = whtp.tile([128, 2 * R], F32R, tag="wht")
                for mh in range(2):
                    ps = miscps.tile([128, R], F32, tag="mp", name="whtps")
                    for fc in range(FC):
                        nc.tensor.matmul(
                            ps[:],
                            r32(w_sb[:, fc * NHID + mh * 128: fc * NHID + (mh + 1) * 128]),
                            r32(xt_sb[:, fc * R:(fc + 1) * R]),
                            start=(fc == 0),
                            stop=(fc == FC - 1),
                        )
                    nc.scalar.copy(wht[:, mh * R:(mh + 1) * R], ps[:])

                src_ps = smallps.tile([1, R], F32, tag="sp", name="srcps")
                for kc in range(2):
                    nc.tensor.matmul(
                        src_ps[0:1, :],
                        r32(a_sb[:, kc:kc + 1]),
                        r32(wht[:, kc * R:(kc + 1) * R]),
                        start=(kc == 0),
                        stop=(kc == 1),
                    )
                dst_ps = smallps.tile([1, R], F32, tag="sp", name="dstps")
                for kc in range(2):
                    nc.tensor.matmul(
                        dst_ps[0:1, :],
                        r32(a_sb[:, 2 + kc:3 + kc]),
                        r32(wht[:, kc * R:(kc + 1) * R]),
                        start=(kc == 0),
                        stop=(kc == 1),
                    )
                # p_i = exp(0.8 * src_i), broadcast along partitions
                src16 = smallp.tile([1, R], F16, tag="src16")
                nc.scalar.activation(src16[:1, :], src_ps[0:1, :], AF.Exp, scale=0.8)
                sb = srcbp.tile([128, R], F16, tag=f"srcb{u}")
                nc.gpsimd.partition_broadcast(sb[:], src16[:1, :])
                srcb[u] = sb
                dst32 = smallp.tile([1, R], F32, tag="dst32")
                nc.scalar.copy(dst32[:1, :], dst_ps[0:1, :])
                nc.sync.dma_start(dstb.ap()[u:u + 1, :], dst32[:1, :])

            # gather dst; arrives partition-innermost -> load c-major + PE-transpose
            nc.gpsimd.collective_compute(
                "AllGather", ALU.bypass, replica_groups=groups,
                ins=[dstb.ap().opt()], outs=[dstg.ap().opt()],
            )

            # ======== z0 (after dst on the cc stream) ========
            ws0_sb = wsbp.tile([128, NHEADS * FC * NHID_S], F32R, tag="ws0")
            nc.sync.dma_start(
                ws0_sb[:].rearrange("p (h c n) -> p h c n", h=NHEADS, c=FC),
                r32(Ws0_d.ap()).rearrange("h (c p) n -> p h c n", p=128),
            )
            for h in range(NHEADS):
                for it in range(IT):
                    ps = miscps.tile([128, NHID_S], F32, tag="mp", name="zps")
                    for fc in range(FC):
                        nc.tensor.matmul(
                            ps[:],
                            r32(xst_sb[:, fc * R + it * 128: fc * R + (it + 1) * 128]),
                            r32(ws0_sb[:, (h * FC + fc) * NHID_S:(h * FC + fc + 1) * NHID_S]),
                            start=(fc == 0),
                            stop=(fc == FC - 1),
                        )
                    st = stagep.tile([128, NHID_S], F16, tag="zstage")
                    nc.scalar.copy(st[:], ps[:])
                    nc.sync.dma_start(
                        zb0.ap()[it * 128:(it + 1) * 128, h * NHID_S:(h + 1) * NHID_S],
                        st[:],
                    )
            nc.gpsimd.collective_compute(
                "AllGather", ALU.bypass, replica_groups=groups,
                ins=[zb0.ap().opt()], outs=[zg0.ap().opt()],
            )
            dst_sb = constp.tile([128, 9 * JC], F32, tag="dst")
            for u in UNITS:
                dsttmp = smallp.tile([32, 128], F32, tag="dsttmp")
                for k in range(NC_):
                    nc.sync.dma_start(
                        dsttmp[k * 4:(k + 1) * 4, :],
                        dstg.ap()[k, u].rearrange("(r p) -> r p", p=128),
                    )
                tp9 = smallps.tile([128, 32], F32, tag="sp", name="dstps2")
                nc.tensor.matmul(
                    tp9[:], dsttmp[:32, :], ident[:32, :32], is_transpose=True
                )
                nc.scalar.copy(dst_sb[:, u * JC:(u + 1) * JC], tp9[:])
            # q_j = exp(0.8 dst_j), b_j = exp(0.2 dst_j)
            q_sb = constp.tile([128, 9 * JC], F32, tag="qsb")
            nc.scalar.activation(q_sb[:], dst_sb[:], AF.Exp, scale=0.8)
            b_sb = constp.tile([128, 9 * JC], F32, tag="bsb")
            nc.scalar.activation(b_sb[:], dst_sb[:], AF.Exp, scale=0.2)

            # ======== helpers ========
            def build_z_slice(zg, ncols_z, tag):
                """Gathered z [N, ncols_z] f16 -> SBUF [128, JC*(ncols_z+1)] + ones col."""
                zt = zsbp.tile([128, JC * (ncols_z + 1)], F16, tag=tag, name=f"zsb_{tag}")
                view = zt[:].rearrange("p (c n) -> p c n", n=ncols_z + 1)
                nc.sync.dma_start(
                    view[:, :, 0:ncols_z],
                    zg.ap().rearrange("(c p) n -> p c n", p=128),
                )
                nc.vector.memset(view[:, :, ncols_z:ncols_z + 1], 1.0)
                return zt

            def build_z_slice_col(zg, col0, ncols_z, tag):
                zt = zsbp.tile([128, JC * (ncols_z + 1)], F16, tag=tag, name=f"zsbc_{tag}")
                view = zt[:].rearrange("p (c n) -> p c n", n=ncols_z + 1)
                nc.sync.dma_start(
                    view[:, :, 0:ncols_z],
                    zg.ap()[:, col0:col0 + ncols_z].rearrange(
                        "(c p) n -> p c n", p=128
                    ),
                )
                nc.vector.memset(view[:, :, ncols_z:ncols_z + 1], 1.0)
                return zt

            def attention(u, z_sb, ncols):
                """acc[it] = attT_chunks(u).T @ [b*z | b] accumulated over chunks."""
                nz = ncols + 1
                # zv = diag(b) @ [z | 1]  (on ACT: copy with per-partition scale)
                zv = zvp.tile([128, JC * nz], F16, tag="zv")
                for c in range(JC):
                    nc.scalar.activation(
                        zv[:, c * nz:(c + 1) * nz],
                        z_sb[:, c * nz:(c + 1) * nz],
                        AF.Copy, bias=0.0,
                        scale=b_sb[:, u * JC + c:u * JC + c + 1],
                    )
                acc = [
                    attps.tile([128, nz], F32, tag=f"attacc{it}", name=f"attacc{it}")
                    for it in range(IT)
                ]
                for g in range(JC // GRP):
                    tt = atttp.tile([128, GRP * R], F16, tag="attt")
                    for cc in range(GRP):
                        c = g * GRP + cc
                        sl = tt[:, cc * R:(cc + 1) * R]
                        nc.vector.tensor_scalar(
                            sl, srcb[u][:], q_sb[:, u * JC + c:u * JC + c + 1],
                            1.0, ALU.mult, ALU.max,
                        )
                    nc.vector.tensor_tensor(
                        tt[:], tt[:], adjm[:, g * GRP * R:(g + 1) * GRP * R], ALU.mult
                    )
                    for cc in range(GRP):
                        c = g * GRP + cc
                        for it in range(IT):
                            nc.tensor.matmul(
                                acc[it][:],
                                tt[:, cc * R + it * 128: cc * R + (it + 1) * 128],
                                zv[:, c * nz:(c + 1) * nz],
                                start=(c == 0),
                                stop=(c == JC - 1),
                            )
                return acc

            def postprocess(acc, ncols, to_hsT=None, to_out=None):
                """hs = elu(num/den);  elu(v) = max(exp(-relu(-v)) - 1, v)."""
                for it in range(IT):
                    rd = postp.tile([128, 1], F32, tag="rd")
                    nc.vector.reciprocal(rd[:], acc[it][:, ncols:ncols + 1])
                    hv = postp.tile([128, ncols], F32, tag="hv")
                    nc.vector.tensor_scalar(
                        hv[:], acc[it][:, 0:ncols], rd[:, 0:1], None, ALU.mult
                    )
                    mn = postp.tile([128, ncols], F32, tag="mn")
                    nc.scalar.activation(mn[:], hv[:], AF.Relu, scale=-1.0)
                    nc.scalar.activation(mn[:], mn[:], AF.Exp, scale=-1.0)
                    elu = postp.tile([128, ncols], F32, tag="elu")
                    nc.vector.scalar_tensor_tensor(
                        elu[:], mn[:], -1.0, hv[:], ALU.add, ALU.max
                    )
                    if to_out is not None:
                        nc.sync.dma_start(
                            to_out.ap()[it * 128:(it + 1) * 128, :], elu[:]
                        )
                    else:
                        tp = smallps.tile([128, 128], F32, tag="sp", name="trps")
                        nc.tensor.matmul(tp[:], elu[:], ident[:], is_transpose=True)
                        nc.scalar.copy(to_hsT[:, it * 128:(it + 1) * 128], tp[:])

            # ======== layer 0 attention (+ pipelined z1 per head) ========
            ws1_sb = wsbp.tile([128, NHEADS * NHID_S], F32R, tag="ws1")
            nc.sync.dma_start(
                ws1_sb[:].rearrange("p (h n) -> p h n", h=NHEADS),
                r32(Ws1_d.ap()).rearrange("h p n -> p h n"),
            )
            hsT = {}
            for h in range(NHEADS):
                z_sb = build_z_slice_col(zg0, h * NHID_S, NHID_S, tag="z")
                acc = attention(2 * h, z_sb, NHID_S)
                ht = hstp.tile([128, R], F32R, tag=f"hsT{h}", name=f"hsT{h}")
                postprocess(acc, NHID_S, to_hsT=ht)
                hsT[h] = ht
                # z1[h] = hs0[h] @ Ws1[h] -> bounce -> gather (overlaps next heads)
                for it in range(IT):
                    ps = miscps.tile([128, NHID_S], F32, tag="mp", name="zps1")
                    nc.tensor.matmul(
                        ps[:],
                        r32(ht[:, it * 128:(it + 1) * 128]),
                        r32(ws1_sb[:, h * NHID_S:(h + 1) * NHID_S]),
                        start=True, stop=True,
                    )
                    st = stagep.tile([128, NHID_S], F16, tag="zstage")
                    nc.scalar.copy(st[:], ps[:])
                    nc.sync.dma_start(
                        zb1[h].ap()[it * 128:(it + 1) * 128, :], st[:]
                    )
                nc.gpsimd.collective_compute(
                    "AllGather", ALU.bypass, replica_groups=groups,
                    ins=[zb1[h].ap().opt()], outs=[zg1[h].ap().opt()],
                )

            # ======== layer 1 attention ========
            for h in range(NHEADS):
                z_sb = build_z_slice(zg1[h], NHID_S, tag="z")
                acc = attention(2 * h + 1, z_sb, NHID_S)
                ht = hstp.tile([128, R], F32R, tag=f"hsT{h}", name=f"hsT1{h}")
                postprocess(acc, NHID_S, to_hsT=ht)
                hsT[h] = ht

            # ======== zo ========
            wso_sb = wsbp.tile([128, NHEADS * OUT_S], F32R, tag="wso")
            nc.sync.dma_start(
                wso_sb[:].rearrange("p (h n) -> p h n", h=NHEADS),
                r32(Wso_d.ap()).rearrange("(h p) n -> p h n", p=128),
            )
            for it in range(IT):
                ps = miscps.tile([128, OUT_S], F32, tag="mp", name="zops")
                for h in range(NHEADS):
                    nc.tensor.matmul(
                        ps[:],
                        r32(hsT[h][:, it * 128:(it + 1) * 128]),
                        r32(wso_sb[:, h * OUT_S:(h + 1) * OUT_S]),
                        start=(h == 0),
                        stop=(h == NHEADS - 1),
                    )
                st = stagep.tile([128, OUT_S], F16, tag="zostage")
                nc.scalar.copy(st[:], ps[:])
                nc.sync.dma_start(zbo.ap()[it * 128:(it + 1) * 128, :], st[:])
            nc.gpsimd.collective_compute(
                "AllGather", ALU.bypass, replica_groups=groups,
                ins=[zbo.ap().opt()], outs=[zgo.ap().opt()],
            )

            # ======== output attention ========
            z_sb = build_z_slice(zgo, OUT_S, tag="z")
            acc = attention(U_OUT, z_sb, OUT_S)
            postprocess(acc, OUT_S, to_out=out_d)

    nc.compile()
    return nc


_CACHE = {}


def _get_nc():
    if "nc" not in _CACHE:
        _CACHE["nc"] = build_program()
    return _CACHE["nc"]


def make_in_maps(x, adj, observation, x_struc, theta, W, a, Ws0, Ws1, Wo, ao, Wso):
    x = np.asarray(x, np.float32)
    adj = np.asarray(adj, np.float32)
    x_struc = np.asarray(x_struc, np.float32)
    in_maps = []
    for k in range(NC_):
        rows = slice(k * R, (k + 1) * R)
        in_maps.append({
            "xT": np.ascontiguousarray(x[rows].T),
            "xsT": np.ascontiguousarray(x_struc[rows].T),
            "adjT": np.ascontiguousarray(adj[rows].T),
            "obs": np.ascontiguousarray(np.asarray(observation, np.int32)[:, rows]),
            "theta": np.asarray(theta, np.float32),
            "W": np.asarray(W, np.float32),
            "a": np.asarray(a, np.float32),
            "Ws0": np.asarray(Ws0, np.float32),
            "Ws1": np.asarray(Ws1, np.float32),
            "Wo": np.asarray(Wo, np.float32),
            "ao": np.asarray(ao, np.float32),
            "Wso": np.asarray(Wso, np.float32),
        })
    return in_maps


def kernel(**inputs) -> np.ndarray:
    from concourse.bass_utils import run_bass_kernel_spmd

    nc = _get_nc()
    in_maps = make_in_maps(**inputs)
    res = run_bass_kernel_spmd(nc, in_maps, list(range(NC_)))
    out = np.concatenate([res.results[k]["out"] for k in range(NC_)], axis=0)
    return out.astype(np.float32)
